# revision 5
# baseline (speedup 1.0000x reference)
"""CenterLoss kernel for 8 Trainium2 NeuronCores (Bass/Tile).

Reference computation:
    label = argmax(predicts, axis=-1)            # [N], N = 32*256 = 8192
    d_n   = ||features_n - centers[label_n]||^2  # [N]
    loss  = (sum_n clip(d_n, EPS, INF) + N*(C-1)*EPS) / N

(The N*(C-1)*EPS term comes from the reference clipping the zeroed
mask-complement entries of the [N, C] masked distance matrix to EPS.)

Sharding: data-parallel over the flattened N axis — 1024 rows per core,
centers replicated. Per core the kernel streams its [1024, 6625] predicts
shard through SBUF in 8 [128, 6625] tiles, computes per-row argmax with the
DVE Max8/FindIndex8 instructions, gathers centers rows with per-partition
indirect DMA, and reduces to per-row clipped squared distances. The host
sums the 8 per-core partial vectors (the scalar "all-reduce") and applies
the EPS correction.
"""

import numpy as np

import concourse.bacc as bacc
import concourse.bass as bass
import concourse.mybir as mybir
from concourse import tile
from concourse.bass_utils import run_bass_kernel_spmd

B, T, D, C = 32, 256, 96, 6625
N = B * T                  # 8192 rows total
NCORES = 8
NS = N // NCORES           # 1024 rows per core
P = 128                    # SBUF partitions
NT = NS // P               # 8 predicts tiles per core
EPS = 1e-7

# test.py toggles these module-level knobs; the grading harness just calls
# kernel(**inputs) and gets the defaults.
TRACE = False
TRACE_KWARGS = {}
LAST_RESULTS = None


def _build():
    nc = bacc.Bacc("TRN2", num_devices=NCORES)
    f32 = mybir.dt.float32
    pred = nc.dram_tensor("predicts", [NS, C], f32, kind="ExternalInput").ap()
    feat = nc.dram_tensor("features", [NS, D], f32, kind="ExternalInput").ap()
    cent = nc.dram_tensor("centers", [C, D], f32, kind="ExternalInput").ap()
    dist = nc.dram_tensor("dists", [P, NT], f32, kind="ExternalOutput").ap()
    labs = nc.dram_tensor("labels", [P, NT], mybir.dt.uint32, kind="ExternalOutput").ap()

    with tile.TileContext(nc) as tc:
        with (
            tc.tile_pool(name="pred", bufs=3) as pp,
            tc.tile_pool(name="small", bufs=2) as sp,
            tc.tile_pool(name="persist", bufs=1) as ps,
        ):
            ftile = ps.tile([P, NT, D], f32)
            nc.sync.dma_start(ftile[:], feat.rearrange("(t p) d -> p t d", p=P))
            labt = ps.tile([P, NT], mybir.dt.uint32)
            ctile = ps.tile([P, NT, D], f32)

            for t in range(NT):
                pt = pp.tile([P, C], f32, tag="pt")
                # SWDGE: the HWDGE direct2d instruction only encodes one sync
                # wait, but slot reuse here needs two (DVE release + DMA lane).
                nc.gpsimd.dma_start(pt[:], pred[t * P : (t + 1) * P, :])
                mx8 = sp.tile([P, 8], f32, tag="mx8")
                idx8 = sp.tile([P, 8], mybir.dt.uint32, tag="idx8")
                nc.vector.max(out=mx8[:], in_=pt[:])
                nc.vector.max_index(out=idx8[:], in_max=mx8[:], in_values=pt[:])
                nc.vector.tensor_copy(labt[:, t : t + 1], idx8[:, 0:1])
                # centers[label] gather: one 384B row per partition
                nc.gpsimd.indirect_dma_start(
                    out=ctile[:, t, :],
                    out_offset=None,
                    in_=cent[:],
                    in_offset=bass.IndirectOffsetOnAxis(ap=labt[:, t : t + 1], axis=0),
                )

            diff = ps.tile([P, NT, D], f32)
            nc.vector.tensor_sub(diff[:], ftile[:], ctile[:])
            nc.vector.tensor_mul(diff[:], diff[:], diff[:])
            d2 = ps.tile([P, NT], f32)
            nc.vector.reduce_sum(d2[:], diff[:], axis=mybir.AxisListType.X)
            nc.vector.tensor_scalar_max(d2[:], d2[:], EPS)
            nc.sync.dma_start(dist[:], d2[:])
            nc.sync.dma_start(labs[:], labt[:])
    nc.compile()
    return nc


def kernel(features, predicts, centers):
    global LAST_RESULTS
    feats = np.ascontiguousarray(np.asarray(features).reshape(N, D), dtype=np.float32)
    preds = np.ascontiguousarray(np.asarray(predicts).reshape(N, C), dtype=np.float32)
    cents = np.ascontiguousarray(np.asarray(centers), dtype=np.float32)

    nc = _build()
    in_maps = [
        {
            "predicts": preds[i * NS : (i + 1) * NS],
            "features": feats[i * NS : (i + 1) * NS],
            "centers": cents,
        }
        for i in range(NCORES)
    ]
    res = run_bass_kernel_spmd(
        nc, in_maps, core_ids=list(range(NCORES)), trace=TRACE, **TRACE_KWARGS
    )
    LAST_RESULTS = res

    total = 0.0
    for r in res.results:
        total += float(r["dists"].astype(np.float64).sum())
    total += float(N) * (C - 1) * EPS
    return np.asarray(total / N, dtype=np.float32)


# revision 7
# speedup vs baseline: 1.2343x; 1.2343x over previous
"""CenterLoss kernel for 8 Trainium2 NeuronCores (Bass/Tile).

Reference computation:
    label = argmax(predicts, axis=-1)            # [N], N = 32*256 = 8192
    d_n   = ||features_n - centers[label_n]||^2  # [N]
    loss  = (sum_n clip(d_n, EPS, INF) + N*(C-1)*EPS) / N

(The N*(C-1)*EPS term comes from the reference clipping the zeroed
mask-complement entries of the [N, C] masked distance matrix to EPS.)

Sharding: data-parallel over the flattened N axis — 1024 rows per core,
centers replicated. Per core the kernel streams its [1024, 6625] predicts
shard through SBUF in 8 [128, 6625] tiles, computes per-row argmax with the
DVE Max8/FindIndex8 instructions, gathers centers rows with per-partition
indirect DMA, and reduces to per-row clipped squared distances. The host
sums the 8 per-core partial vectors (the scalar "all-reduce") and applies
the EPS correction.
"""

import numpy as np

import concourse.bacc as bacc
import concourse.bass as bass
import concourse.mybir as mybir
from concourse import tile
from concourse.bass_utils import run_bass_kernel_spmd

B, T, D, C = 32, 256, 96, 6625
N = B * T                  # 8192 rows total
NCORES = 8
NS = N // NCORES           # 1024 rows per core
P = 128                    # SBUF partitions
NT = NS // P               # 8 predicts tiles per core
NCH = 53                   # chunks per row for hierarchical argmax
CW = 125                   # chunk width (53 * 125 = 6625)
EPS = 1e-7

# test.py toggles these module-level knobs; the grading harness just calls
# kernel(**inputs) and gets the defaults.
TRACE = False
TRACE_KWARGS = {}
LAST_RESULTS = None


def _build():
    nc = bacc.Bacc("TRN2", num_devices=NCORES)
    f32 = mybir.dt.float32
    pred = nc.dram_tensor("predicts", [NS, C], f32, kind="ExternalInput").ap()
    feat = nc.dram_tensor("features", [NS, D], f32, kind="ExternalInput").ap()
    cent = nc.dram_tensor("centers", [C, D], f32, kind="ExternalInput").ap()
    dist = nc.dram_tensor("dists", [P, NT], f32, kind="ExternalOutput").ap()
    labs = nc.dram_tensor("labels", [P, NT], mybir.dt.uint32, kind="ExternalOutput").ap()

    u32 = mybir.dt.uint32
    # flat chunk view for the winning-chunk regather: row r, chunk k lives at
    # predflat[r * NCH + k, :]
    predflat = pred.rearrange("n (k q) -> (n k) q", q=CW)

    with tile.TileContext(nc) as tc:
        with (
            tc.tile_pool(name="pred", bufs=3) as pp,
            tc.tile_pool(name="small", bufs=3) as sp,
            tc.tile_pool(name="persist", bufs=1) as ps,
        ):
            ftile = ps.tile([P, NT, D], f32)
            nc.sync.dma_start(ftile[:], feat.rearrange("(t p) d -> p t d", p=P))
            labt = ps.tile([P, NT], u32)
            ctile = ps.tile([P, NT, D], f32)
            gath = ps.tile([P, NT, CW], f32)
            offs = ps.tile([P, NT], u32)
            iotas = ps.tile([P, NT], mybir.dt.int32)
            for t in range(NT):
                # base chunk index of row (t*P + p): (t*P + p) * NCH
                nc.gpsimd.iota(
                    iotas[:, t : t + 1], pattern=[[1, 1]], base=t * P * NCH,
                    channel_multiplier=NCH,
                )

            for t in range(NT):
                pt = pp.tile([P, C], f32, tag="pt")
                # SWDGE: the HWDGE direct2d instruction only encodes one sync
                # wait, but slot reuse here needs two (DVE release + DMA lane).
                nc.gpsimd.dma_start(pt[:], pred[t * P : (t + 1) * P, :])
                # hierarchical argmax: one full pass for per-chunk maxes, then
                # index work on the 53 chunk maxes + the 125-wide winning chunk
                cm = sp.tile([P, NCH], f32, tag="cm")
                nc.vector.reduce_max(
                    cm[:], pt[:].rearrange("p (k q) -> p k q", q=CW),
                    axis=mybir.AxisListType.X,
                )
                top8 = sp.tile([P, 8], f32, tag="top8")
                cidx8 = sp.tile([P, 8], u32, tag="cidx8")
                nc.vector.max(out=top8[:], in_=cm[:])
                nc.vector.max_index(out=cidx8[:], in_max=top8[:], in_values=cm[:])
                nc.vector.tensor_add(
                    offs[:, t : t + 1], iotas[:, t : t + 1], cidx8[:, 0:1]
                )
                nc.gpsimd.indirect_dma_start(
                    out=gath[:, t, :],
                    out_offset=None,
                    in_=predflat,
                    in_offset=bass.IndirectOffsetOnAxis(ap=offs[:, t : t + 1], axis=0),
                )
                widx8 = sp.tile([P, 8], u32, tag="widx8")
                nc.vector.max_index(
                    out=widx8[:], in_max=top8[:], in_values=gath[:, t, :]
                )
                # label = cidx * CW + widx
                nc.vector.tensor_scalar(
                    labt[:, t : t + 1], cidx8[:, 0:1], float(CW), None,
                    op0=mybir.AluOpType.mult,
                )
                nc.vector.tensor_add(
                    labt[:, t : t + 1], labt[:, t : t + 1], widx8[:, 0:1]
                )
                # centers[label] gather: one 384B row per partition
                nc.gpsimd.indirect_dma_start(
                    out=ctile[:, t, :],
                    out_offset=None,
                    in_=cent[:],
                    in_offset=bass.IndirectOffsetOnAxis(ap=labt[:, t : t + 1], axis=0),
                )

            diff = ps.tile([P, NT, D], f32)
            nc.vector.tensor_sub(diff[:], ftile[:], ctile[:])
            d2 = ps.tile([P, NT], f32)
            sq = ps.tile([P, NT, D], f32)
            for t in range(NT):
                nc.scalar.activation(
                    sq[:, t, :], diff[:, t, :], mybir.ActivationFunctionType.Square,
                    accum_out=d2[:, t : t + 1],
                )
            nc.vector.tensor_scalar_max(d2[:], d2[:], EPS)
            nc.sync.dma_start(dist[:], d2[:])
            nc.sync.dma_start(labs[:], labt[:])
    nc.compile()
    return nc


def kernel(features, predicts, centers):
    global LAST_RESULTS
    feats = np.ascontiguousarray(np.asarray(features).reshape(N, D), dtype=np.float32)
    preds = np.ascontiguousarray(np.asarray(predicts).reshape(N, C), dtype=np.float32)
    cents = np.ascontiguousarray(np.asarray(centers), dtype=np.float32)

    nc = _build()
    in_maps = [
        {
            "predicts": preds[i * NS : (i + 1) * NS],
            "features": feats[i * NS : (i + 1) * NS],
            "centers": cents,
        }
        for i in range(NCORES)
    ]
    res = run_bass_kernel_spmd(
        nc, in_maps, core_ids=list(range(NCORES)), trace=TRACE, **TRACE_KWARGS
    )
    LAST_RESULTS = res

    total = 0.0
    for r in res.results:
        total += float(r["dists"].astype(np.float64).sum())
    total += float(N) * (C - 1) * EPS
    return np.asarray(total / N, dtype=np.float32)
